# revision 2
# baseline (speedup 1.0000x reference)
"""CIDER loss Trainium2 kernel (8 NeuronCores, data-parallel over batch).

Math (reference):
  logits = (z @ mu.T) / T          # [B, C],  T = 0.1
  pos    = logits[b, target[b]]
  lse    = logsumexp(logits, axis=1)
  loss_comp = mean(lse - pos)
  sim    = (mu @ mu.T) / T with diag masked to -inf
  loss_dis  = mean(log(1/(C-1)) + logsumexp(sim, axis=1))
  loss = ALPHA * loss_dis + LAMDA * loss_comp

v2 design (per core, B_SH = 8192 rows = 64 tiles of 128):
  - PE: logits10 tile = zT_tile.T @ (mu.T*10)  [128,1000] f32 in PSUM.
    Two tiles share one [128,2048] PSUM group tile (4 banks, 2 bufs).
  - DVE: one merged tensor_reduce(max, negate) per GROUP ([128,2,1000]
    3D AP -> [128,2]), amortizing the PSUM op overhead + semaphores.
  - ACT: exp(l + nm) with accum_out -> per-tile row sums s_cols.
    lse = ln(s) - nm computed once at the end over [128,65].
  - pos: loss only needs SUM(pos) -> two big bf16 stt ops
    (zn*10)⊙mug with accum_out (2x DVE mode), mug = mu[target] rows
    pre-gathered on host (pure input indexing; no gpsimd gather).
  - Dispersion rides the same pipeline as tile #64 (125 rows): the diag
    mask is added via an extra accumulating matmul pair (Asel.T@Bmask),
    costing only idle PE cycles.
  - Host sums the per-core partial scalars.
"""
import sys

if "/opt/trn_rl_repo" not in sys.path:
    sys.path.insert(0, "/opt/trn_rl_repo")

from contextlib import ExitStack

import numpy as np

import concourse.bass as bass
import concourse.tile as tile
from concourse import bacc, mybir
from concourse.bass_utils import run_bass_kernel_spmd

N_CORES = 8
B, D, C = 65536, 128, 1000
B_SH = B // N_CORES            # 8192 rows per core
NT = B_SH // 128               # 64 tiles of 128 rows
CD = C // N_CORES              # dispersion rows per core (125)
SCALE = 10.0                   # 1 / T
ALPHA, LAMDA = 1.0, 2.0
F32 = mybir.dt.float32
BF16 = mybir.dt.bfloat16
AX = mybir.AxisListType
ALU = mybir.AluOpType
ACTF = mybir.ActivationFunctionType

NZCH = 2                       # zn/mug DMA+stt chunks (big, on ACT ring)
ZCOLS = (NT * 128) // NZCH     # 4096 cols per chunk
NTCH = 8                       # zT DMA chunks (sync ring)
TPC = NT // NTCH               # tiles per zT chunk


def _build_program():
    nc = bacc.Bacc("TRN2", target_bir_lowering=False, debug=False,
                   num_devices=N_CORES)
    t = {}
    t["zT"] = nc.dram_tensor("zT", [D, B_SH], BF16, kind="ExternalInput").ap()
    t["zn"] = nc.dram_tensor("zn", [128, NT * 128], BF16,
                             kind="ExternalInput").ap()
    t["mug"] = nc.dram_tensor("mug", [128, NT * 128], BF16,
                              kind="ExternalInput").ap()
    t["muTs"] = nc.dram_tensor("muTs", [D, C], BF16, kind="ExternalInput").ap()
    t["muTd"] = nc.dram_tensor("muTd", [D, CD], BF16,
                               kind="ExternalInput").ap()
    t["asel"] = nc.dram_tensor("asel", [128, CD], BF16,
                               kind="ExternalInput").ap()
    t["bmask"] = nc.dram_tensor("bmask", [128, C], BF16,
                                kind="ExternalInput").ap()
    t["out"] = nc.dram_tensor("out", [1, 3], F32, kind="ExternalOutput").ap()

    with tile.TileContext(nc) as tc, ExitStack() as ctx:
        _build_tile_program(tc, ctx, t)
    nc.compile()
    return nc


def _build_tile_program(tc, ctx, t):
    nc = tc.nc
    singles = ctx.enter_context(tc.tile_pool(name="singles", bufs=1))
    scr_pool = ctx.enter_context(tc.tile_pool(name="scr", bufs=2))
    pp_pool = ctx.enter_context(tc.tile_pool(name="pp", bufs=2))
    ps_pool = ctx.enter_context(tc.tile_pool(name="ps", bufs=2, space="PSUM"))

    # --- DMA: sync ring carries the matmul-critical loads in priority
    # order; the two zn/mug pairs ride the ACT ring (4 triggers only).
    muTs = singles.tile([D, C], BF16)
    nc.sync.dma_start(muTs[:], t["muTs"][:, :])
    zT_ch = []
    for c in range(NTCH):
        zt = singles.tile([D, TPC * 128], BF16, tag=f"zTc{c}")
        nc.sync.dma_start(zt[:], t["zT"][:, c * TPC * 128:(c + 1) * TPC * 128])
        zT_ch.append(zt)
    muTd = singles.tile([D, CD], BF16)
    nc.sync.dma_start(muTd[:], t["muTd"][:, :])
    asel = singles.tile([128, CD], BF16)
    nc.sync.dma_start(asel[:], t["asel"][:, :])
    bmask = singles.tile([128, C], BF16)
    nc.sync.dma_start(bmask[:], t["bmask"][:, :])

    zn = singles.tile([128, NT * 128], BF16)
    mug = singles.tile([128, NT * 128], BF16)
    for c in range(NZCH):
        sl = slice(c * ZCOLS, (c + 1) * ZCOLS)
        nc.scalar.dma_start(zn[:, sl], t["zn"][:, sl])
        nc.scalar.dma_start(mug[:, sl], t["mug"][:, sl])

    ones = singles.tile([128, 1], F32)
    nc.vector.memset(ones[:], 1.0)
    s_cols = singles.tile([128, NT + 1], F32)
    nc.vector.memset(s_cols[:], 1.0)
    nm_cols = singles.tile([128, NT + 1], F32)
    nc.vector.memset(nm_cols[:], 0.0)
    pos_cols = singles.tile([128, NZCH], F32)
    nc.vector.memset(pos_cols[:], 0.0)

    # --- main loop over 32 groups of 2 row-tiles, then dispersion.
    def emit_group(g):
        ps2 = ps_pool.tile([128, 2048], F32, tag="ps")
        for h in range(2):
            j = 2 * g + h
            c, jj = j // TPC, j % TPC
            lhs = zT_ch[c][:, jj * 128:(jj + 1) * 128]
            base = h * 1024
            nc.tensor.matmul(ps2[:, base:base + 512], lhs, muTs[:, 0:512],
                             start=True, stop=True)
            nc.tensor.matmul(ps2[:, base + 512:base + 1000], lhs,
                             muTs[:, 512:1000], start=True, stop=True)
        red_in = ps2[:].rearrange("p (g c) -> p g c", g=2)[:, :, 0:1000]
        nc.vector.tensor_reduce(out=nm_cols[:, 2 * g:2 * g + 2], in_=red_in,
                                axis=AX.X, op=ALU.max, negate=True)
        for h in range(2):
            j = 2 * g + h
            base = h * 1024
            scr = scr_pool.tile([128, C], BF16, tag="scr")
            nc.scalar.activation(out=scr[:], in_=ps2[:, base:base + 1000],
                                 func=ACTF.Exp, bias=nm_cols[:, j:j + 1],
                                 scale=1.0, accum_out=s_cols[:, j:j + 1])

    def emit_pos_chunk(c):
        sl = slice(c * ZCOLS, (c + 1) * ZCOLS)
        pp = pp_pool.tile([128, ZCOLS], BF16, tag="pp")
        nc.vector.scalar_tensor_tensor(
            out=pp[:], in0=zn[:, sl], scalar=SCALE, in1=mug[:, sl],
            op0=ALU.mult, op1=ALU.mult, accum_out=pos_cols[:, c:c + 1])

    for g in range(NT // 2):
        emit_group(g)
        # pos chunks interleave once their (ACT-ring) DMA has landed.
        if g == 4:
            emit_pos_chunk(0)
        elif g == 8:
            emit_pos_chunk(1)

    # Dispersion tile (#64): sim rows k*125..k*125+124, diag masked via an
    # extra accumulating matmul pair instead of a DVE add.
    psd = ps_pool.tile([CD, 2048], F32, tag="ps")
    nc.tensor.matmul(psd[:, 0:512], muTd[:, :], muTs[:, 0:512],
                     start=True, stop=False)
    nc.tensor.matmul(psd[:, 0:512], asel[:, :], bmask[:, 0:512],
                     start=False, stop=True)
    nc.tensor.matmul(psd[:, 512:1000], muTd[:, :], muTs[:, 512:1000],
                     start=True, stop=False)
    nc.tensor.matmul(psd[:, 512:1000], asel[:, :], bmask[:, 512:1000],
                     start=False, stop=True)
    nc.vector.tensor_reduce(out=nm_cols[0:CD, NT:NT + 1],
                            in_=psd[:, 0:1000], axis=AX.X, op=ALU.max,
                            negate=True)
    scrd = scr_pool.tile([128, C], BF16, tag="scr")
    nc.scalar.activation(out=scrd[0:CD, :], in_=psd[:, 0:1000], func=ACTF.Exp,
                         bias=nm_cols[0:CD, NT:NT + 1], scale=1.0,
                         accum_out=s_cols[0:CD, NT:NT + 1])

    # --- tail: lse rows = ln(s) - nm; partition-sums via PE ones-trick.
    ln_cols = singles.tile([128, NT + 1], F32)
    nc.scalar.activation(out=ln_cols[:], in_=s_cols[:], func=ACTF.Ln)
    contrib = singles.tile([128, NT + 1], F32)
    nc.vector.tensor_sub(contrib[:], ln_cols[:], nm_cols[:])
    comp_red = singles.tile([128, 1], F32)
    nc.vector.tensor_reduce(out=comp_red[:], in_=contrib[:, 0:NT], axis=AX.X,
                            op=ALU.add)
    pos_red = singles.tile([128, 1], F32)
    nc.vector.tensor_reduce(out=pos_red[:], in_=pos_cols[:], axis=AX.X,
                            op=ALU.add)

    ps_c = ps_pool.tile([1, 1], F32, tag="ps")
    nc.tensor.matmul(ps_c[0:1, 0:1], comp_red[:, 0:1], ones[:, 0:1],
                     start=True, stop=True)
    ps_p = ps_pool.tile([1, 1], F32, tag="ps")
    nc.tensor.matmul(ps_p[0:1, 0:1], pos_red[:, 0:1], ones[:, 0:1],
                     start=True, stop=True)
    ps_d = ps_pool.tile([1, 1], F32, tag="ps")
    nc.tensor.matmul(ps_d[0:1, 0:1], contrib[0:CD, NT:NT + 1],
                     ones[0:CD, 0:1], start=True, stop=True)
    out_sb = singles.tile([1, 3], F32)
    nc.vector.tensor_copy(out_sb[0:1, 0:1], ps_c[0:1, 0:1])
    nc.vector.tensor_copy(out_sb[0:1, 1:2], ps_p[0:1, 0:1])
    nc.vector.tensor_copy(out_sb[0:1, 2:3], ps_d[0:1, 0:1])
    nc.sync.dma_start(t["out"][:, :], out_sb[:])


_NC_CACHE = {}


def _get_program():
    if "nc" not in _NC_CACHE:
        _NC_CACHE["nc"] = _build_program()
    return _NC_CACHE["nc"]


def make_in_maps(z, target, mu):
    import ml_dtypes
    bf16 = ml_dtypes.bfloat16
    z = np.ascontiguousarray(np.asarray(z, dtype=np.float32))
    mu = np.ascontiguousarray(np.asarray(mu, dtype=np.float32))
    target = np.asarray(target).astype(np.int64)
    muTs = np.ascontiguousarray((mu.T * np.float32(SCALE)).astype(bf16))
    muT_bf = np.ascontiguousarray(mu.T.astype(bf16))           # [128, 1000]
    asel = np.zeros((128, CD), dtype=np.float32)
    asel[np.arange(CD), np.arange(CD)] = 1.0
    asel = asel.astype(bf16)
    mu_rows = mu.astype(bf16)                                  # [1000, 128]
    in_maps = []
    for k in range(N_CORES):
        zs = z[k * B_SH:(k + 1) * B_SH]                        # [8192, 128]
        zT = np.ascontiguousarray(zs.T.astype(bf16))           # [128, 8192]
        zn = np.ascontiguousarray(
            zs.reshape(NT, 128, D).transpose(1, 0, 2)
            .reshape(128, NT * D).astype(bf16))
        ts = target[k * B_SH:(k + 1) * B_SH]
        mug = np.ascontiguousarray(
            mu_rows[ts].reshape(NT, 128, D).transpose(1, 0, 2)
            .reshape(128, NT * D))
        bmask = np.zeros((128, C), dtype=np.float32)
        bmask[np.arange(CD), k * CD + np.arange(CD)] = np.float32(-1e30)
        in_maps.append({
            "zT": zT,
            "zn": zn,
            "mug": mug,
            "muTs": muTs,
            "muTd": np.ascontiguousarray(muT_bf[:, k * CD:(k + 1) * CD]),
            "asel": asel,
            "bmask": bmask.astype(bf16),
        })
    return in_maps


def combine_outputs(results):
    outs = np.stack([np.asarray(r["out"]).reshape(3) for r in results])
    # out = [sum(lse10 rows), sum(pos10), sum(lse_dis rows)]
    comp_total = (outs[:, 0].astype(np.float64)
                  - outs[:, 1].astype(np.float64)).sum()
    dis_total = outs[:, 2].astype(np.float64).sum()
    loss_comp = comp_total / B
    loss_dis = np.log(1.0 / (C - 1)) + dis_total / C
    return np.array(ALPHA * loss_dis + LAMDA * loss_comp, dtype=np.float32)


def run_on_hw(z, target, mu, trace=False):
    nc = _get_program()
    in_maps = make_in_maps(z, target, mu)
    res = run_bass_kernel_spmd(nc, in_maps, core_ids=list(range(N_CORES)),
                               trace=trace)
    return combine_outputs(res.results), res


def kernel(z, target, mu):
    out, _ = run_on_hw(z, target, mu, trace=False)
    return out


# revision 4
# speedup vs baseline: 1.1344x; 1.1344x over previous
"""CIDER loss Trainium2 kernel (8 NeuronCores, data-parallel over batch).

Math (reference):
  logits = (z @ mu.T) / T          # [B, C],  T = 0.1
  pos    = logits[b, target[b]]
  lse    = logsumexp(logits, axis=1)
  loss_comp = mean(lse - pos)
  sim    = (mu @ mu.T) / T with diag masked to -inf
  loss_dis  = mean(log(1/(C-1)) + logsumexp(sim, axis=1))
  loss = ALPHA * loss_dis + LAMDA * loss_comp

v3 design (per core, B_SH = 8192 rows = 64 tiles of 128):
  - One [128,4096] f32 PSUM tile = 4 manually rotated 1024-col regions
    (subtile dep tracking orders matmul/reduce/exp per region).
  - PE: logits10 tile = zT_tile.T @ (mu.T*10) -> region j%4.
  - DVE: one merged tensor_reduce(max, negate) per PAIR of regions
    ([128,2,1000] 3D AP -> nm_cols[:, j:j+2]), amortizing PSUM op
    overhead + semaphores.
  - ACT: exp(l + nm) with accum_out -> per-tile row sums s_cols.
    lse rows = ln(s) - nm computed once at the end over [128,65].
  - pos: loss only needs SUM(pos); tensor_tensor_reduce over bf16
    zn ⊙ (10*mu[target]) chunks accumulates per-partition partials.
    mu[target] rows are pre-gathered on host (pure input indexing).
  - Dispersion rides the same pipeline as tile #64 (125 rows); diag
    mask added via an extra accumulating matmul pair (Asel.T@Bmask).
  - Output = [128,4] per-partition partials; host does the final sums.
"""
import sys

if "/opt/trn_rl_repo" not in sys.path:
    sys.path.insert(0, "/opt/trn_rl_repo")

from contextlib import ExitStack

import numpy as np

import concourse.bass as bass
import concourse.tile as tile
from concourse import bacc, mybir
from concourse.bass_utils import run_bass_kernel_spmd

N_CORES = 8
B, D, C = 65536, 128, 1000
B_SH = B // N_CORES            # 8192 rows per core
NT = B_SH // 128               # 64 tiles of 128 rows
CD = C // N_CORES              # dispersion rows per core (125)
SCALE = 10.0                   # 1 / T
ALPHA, LAMDA = 1.0, 2.0
F32 = mybir.dt.float32
BF16 = mybir.dt.bfloat16
AX = mybir.AxisListType
ALU = mybir.AluOpType
ACTF = mybir.ActivationFunctionType

NZCH = 8                       # zn/mug stt chunks
ZCOLS = (NT * 128) // NZCH     # 1024 cols per chunk
ZT_CHUNKS = [2, 6] + [8] * 7   # zT DMA chunk sizes in tiles (fast ramp)


def _build_program():
    nc = bacc.Bacc("TRN2", target_bir_lowering=False, debug=False,
                   num_devices=N_CORES)
    t = {}
    t["zT"] = nc.dram_tensor("zT", [D, B_SH], BF16, kind="ExternalInput").ap()
    t["zn"] = nc.dram_tensor("zn", [128, NT * 128], BF16,
                             kind="ExternalInput").ap()
    t["mug"] = nc.dram_tensor("mug", [128, NT * 128], BF16,
                              kind="ExternalInput").ap()
    t["muTs"] = nc.dram_tensor("muTs", [D, C], BF16, kind="ExternalInput").ap()
    t["muTd"] = nc.dram_tensor("muTd", [D, CD], BF16,
                               kind="ExternalInput").ap()
    t["asel"] = nc.dram_tensor("asel", [128, CD], BF16,
                               kind="ExternalInput").ap()
    t["bmask"] = nc.dram_tensor("bmask", [128, C], BF16,
                                kind="ExternalInput").ap()
    t["out"] = nc.dram_tensor("out", [128, 4], F32, kind="ExternalOutput").ap()

    with tile.TileContext(nc) as tc, ExitStack() as ctx:
        _build_tile_program(tc, ctx, t)
    nc.compile()
    return nc


def _build_tile_program(tc, ctx, t):
    nc = tc.nc
    singles = ctx.enter_context(tc.tile_pool(name="singles", bufs=1))
    scr_pool = ctx.enter_context(tc.tile_pool(name="scr", bufs=2))
    pp_pool = ctx.enter_context(tc.tile_pool(name="pp", bufs=2))
    ps_pool = ctx.enter_context(tc.tile_pool(name="ps", bufs=1, space="PSUM"))

    # --- DMA: sync ring carries the matmul-critical loads in priority
    # order; zn/mug ride the ACT ring (4 triggers only).
    muTs = singles.tile([D, C], BF16)
    nc.sync.dma_start(muTs[:], t["muTs"][:, :])
    zT = singles.tile([D, B_SH], BF16)
    pos0 = 0
    for ntile in ZT_CHUNKS:
        sl = slice(pos0 * 128, (pos0 + ntile) * 128)
        nc.sync.dma_start(zT[:, sl], t["zT"][:, sl])
        pos0 += ntile
    muTd = singles.tile([D, CD], BF16)
    nc.sync.dma_start(muTd[:], t["muTd"][:, :])
    asel = singles.tile([128, CD], BF16)
    nc.sync.dma_start(asel[:], t["asel"][:, :])
    bmask = singles.tile([128, C], BF16)
    nc.sync.dma_start(bmask[:], t["bmask"][:, :])

    zn = singles.tile([128, NT * 128], BF16)
    mug = singles.tile([128, NT * 128], BF16)
    for c in range(2):
        sl = slice(c * 4096, (c + 1) * 4096)
        nc.scalar.dma_start(zn[:, sl], t["zn"][:, sl])
        nc.scalar.dma_start(mug[:, sl], t["mug"][:, sl])

    s_cols = singles.tile([128, NT + 1], F32)
    nc.vector.memset(s_cols[:], 1.0)
    nm_cols = singles.tile([128, NT + 1], F32)
    nc.vector.memset(nm_cols[:], 0.0)
    pos_cols = singles.tile([128, NZCH], F32)
    nc.vector.memset(pos_cols[:], 0.0)
    out_sb = singles.tile([128, 4], F32)
    nc.vector.memset(out_sb[:], 0.0)

    P = ps_pool.tile([128, 4096], F32)

    def emit_mm(j):
        r = (j % 4) * 1024
        lhs = zT[:, j * 128:(j + 1) * 128]
        nc.tensor.matmul(P[:, r:r + 512], lhs, muTs[:, 0:512],
                         start=True, stop=True)
        nc.tensor.matmul(P[:, r + 512:r + 1000], lhs, muTs[:, 512:1000],
                         start=True, stop=True)

    def emit_red_pair(j):
        r = (j % 4) * 1024
        red_in = P[:, r:r + 2048].rearrange("p (g c) -> p g c", g=2)
        nc.vector.tensor_reduce(out=nm_cols[:, j:j + 2],
                                in_=red_in[:, :, 0:1000],
                                axis=AX.X, op=ALU.max, negate=True)

    def emit_exp(j):
        r = (j % 4) * 1024
        scr = scr_pool.tile([128, C], BF16, tag="scr")
        nc.scalar.activation(out=scr[:], in_=P[:, r:r + 1000],
                             func=ACTF.Exp, bias=nm_cols[:, j:j + 1],
                             scale=1.0, accum_out=s_cols[:, j:j + 1])

    def emit_pos_chunk(c):
        sl = slice(c * ZCOLS, (c + 1) * ZCOLS)
        pp = pp_pool.tile([128, ZCOLS], BF16, tag="pp")
        nc.vector.scalar_tensor_tensor(
            out=pp[:], in0=zn[:, sl], scalar=1.0, in1=mug[:, sl],
            op0=ALU.mult, op1=ALU.mult, accum_out=pos_cols[:, c:c + 1])

    # software-pipelined main loop: mm runs ~2 tiles ahead of red/exp.
    emit_mm(0)
    emit_mm(1)
    for j in range(0, NT, 2):
        if j + 2 < NT:
            emit_mm(j + 2)
        if j + 3 < NT:
            emit_mm(j + 3)
        emit_red_pair(j)
        emit_exp(j)
        emit_exp(j + 1)
        if j >= 16 and j % 6 == 0 and (j - 16) // 6 < NZCH:
            emit_pos_chunk((j - 16) // 6)

    # Dispersion tile (#64) in region 0: diag mask via extra matmul pair.
    nc.tensor.matmul(P[0:CD, 0:512], muTd[:, :], muTs[:, 0:512],
                     start=True, stop=False)
    nc.tensor.matmul(P[0:CD, 0:512], asel[:, :], bmask[:, 0:512],
                     start=False, stop=True)
    nc.tensor.matmul(P[0:CD, 512:1000], muTd[:, :], muTs[:, 512:1000],
                     start=True, stop=False)
    nc.tensor.matmul(P[0:CD, 512:1000], asel[:, :], bmask[:, 512:1000],
                     start=False, stop=True)
    nc.vector.tensor_reduce(out=nm_cols[0:CD, NT:NT + 1],
                            in_=P[0:CD, 0:1000], axis=AX.X, op=ALU.max,
                            negate=True)
    scrd = scr_pool.tile([128, C], BF16, tag="scr")
    nc.scalar.activation(out=scrd[0:CD, :], in_=P[0:CD, 0:1000],
                         func=ACTF.Exp, bias=nm_cols[0:CD, NT:NT + 1],
                         scale=1.0, accum_out=s_cols[0:CD, NT:NT + 1])

    # --- tail: lse rows = ln(s) - nm; ship per-partition partials.
    ln_cols = singles.tile([128, NT + 1], F32)
    nc.scalar.activation(out=ln_cols[:], in_=s_cols[:], func=ACTF.Ln)
    contrib = singles.tile([128, NT + 1], F32)
    nc.vector.tensor_sub(contrib[:], ln_cols[:], nm_cols[:])
    nc.vector.tensor_reduce(out=out_sb[:, 0:1], in_=contrib[:, 0:NT],
                            axis=AX.X, op=ALU.add)
    nc.vector.tensor_reduce(out=out_sb[:, 1:2], in_=pos_cols[:],
                            axis=AX.X, op=ALU.add)
    nc.vector.tensor_copy(out_sb[0:CD, 2:3], contrib[0:CD, NT:NT + 1])
    nc.sync.dma_start(t["out"][:, :], out_sb[:])


_NC_CACHE = {}


def _get_program():
    if "nc" not in _NC_CACHE:
        _NC_CACHE["nc"] = _build_program()
    return _NC_CACHE["nc"]


def make_in_maps(z, target, mu):
    import ml_dtypes
    bf16 = ml_dtypes.bfloat16
    z = np.ascontiguousarray(np.asarray(z, dtype=np.float32))
    mu = np.ascontiguousarray(np.asarray(mu, dtype=np.float32))
    target = np.asarray(target).astype(np.int64)
    muTs = np.ascontiguousarray((mu.T * np.float32(SCALE)).astype(bf16))
    muT_bf = np.ascontiguousarray(mu.T.astype(bf16))           # [128, 1000]
    asel = np.zeros((128, CD), dtype=np.float32)
    asel[np.arange(CD), np.arange(CD)] = 1.0
    asel = asel.astype(bf16)
    mu_rows10 = (mu * np.float32(SCALE)).astype(bf16)          # [1000, 128]
    in_maps = []
    for k in range(N_CORES):
        zs = z[k * B_SH:(k + 1) * B_SH]                        # [8192, 128]
        zT = np.ascontiguousarray(zs.T.astype(bf16))           # [128, 8192]
        zn = np.ascontiguousarray(
            zs.reshape(NT, 128, D).transpose(1, 0, 2)
            .reshape(128, NT * D).astype(bf16))
        ts = target[k * B_SH:(k + 1) * B_SH]
        mug = np.ascontiguousarray(
            mu_rows10[ts].reshape(NT, 128, D).transpose(1, 0, 2)
            .reshape(128, NT * D))
        bmask = np.zeros((128, C), dtype=np.float32)
        bmask[np.arange(CD), k * CD + np.arange(CD)] = np.float32(-1e30)
        in_maps.append({
            "zT": zT,
            "zn": zn,
            "mug": mug,
            "muTs": muTs,
            "muTd": np.ascontiguousarray(muT_bf[:, k * CD:(k + 1) * CD]),
            "asel": asel,
            "bmask": bmask.astype(bf16),
        })
    return in_maps


def combine_outputs(results):
    outs = np.stack([np.asarray(r["out"]).astype(np.float64)
                     for r in results])                        # [8,128,4]
    comp_total = outs[:, :, 0].sum() - outs[:, :, 1].sum()
    dis_total = outs[:, :, 2].sum()
    loss_comp = comp_total / B
    loss_dis = np.log(1.0 / (C - 1)) + dis_total / C
    return np.array(ALPHA * loss_dis + LAMDA * loss_comp, dtype=np.float32)


def run_on_hw(z, target, mu, trace=False):
    nc = _get_program()
    in_maps = make_in_maps(z, target, mu)
    res = run_bass_kernel_spmd(nc, in_maps, core_ids=list(range(N_CORES)),
                               trace=trace)
    return combine_outputs(res.results), res


def kernel(z, target, mu):
    out, _ = run_on_hw(z, target, mu, trace=False)
    return out


# revision 9
# speedup vs baseline: 1.4432x; 1.2722x over previous
"""CIDER loss Trainium2 kernel (8 NeuronCores, data-parallel over batch).

Math (reference):
  logits = (z @ mu.T) / T          # [B, C],  T = 0.1
  pos    = logits[b, target[b]]
  lse    = logsumexp(logits, axis=1)
  loss_comp = mean(lse - pos)
  sim    = (mu @ mu.T) / T with diag masked to -inf
  loss_dis  = mean(log(1/(C-1)) + logsumexp(sim, axis=1))
  loss = ALPHA * loss_dis + LAMDA * loss_comp

v4 design (per core, B_SH = 8192 rows = 64 tiles of 128):
  - One [128,4096] f32 PSUM tile = 4 manually rotated 1024-col regions
    (subtile dep tracking orders matmul/reduce/exp per region).
  - PE: logits10 tile = zT_tile.T @ (mu.T*10) -> region slot%4.
  - DVE: per-tile tensor_reduce(max, negate) -> nm_cols column.
  - ACT: exp(l + nm) with accum_out -> per-tile row sums s_cols.
    lse rows = ln(s) - nm with ln via the exponent-bits trick (one DVE
    op, < 0.03 nats, mean-zero) -- no second ACT table load.
  - pos: loss only needs SUM(pos) = <zT, mugT> elementwise (transposed
    layout reuses zT; no separate zn upload). mugT = (10*mu[target]).T
    pre-gathered on host (pure input indexing). stt chunks w/ accum.
  - Dispersion rides the pipeline as slot 26 (125 rows); diag mask via
    an extra accumulating matmul pair (Asel.T@Bmask) on idle PE cycles.
  - All loads on the sync HWDGE ring, dma_starts emitted interleaved
    with compute so semaphore wait targets stay small.
  - Output = [128,4] per-partition partials; host does the final sums.
"""
import sys

if "/opt/trn_rl_repo" not in sys.path:
    sys.path.insert(0, "/opt/trn_rl_repo")

from contextlib import ExitStack

import numpy as np

import concourse.bass as bass
import concourse.tile as tile
from concourse import bacc, mybir
from concourse.bass_utils import run_bass_kernel_spmd

N_CORES = 8
B, D, C = 65536, 128, 1000
B_SH = B // N_CORES            # 8192 rows per core
NT = B_SH // 128               # 64 tiles of 128 rows
CD = C // N_CORES              # dispersion rows per core (125)
SCALE = 10.0                   # 1 / T
ALPHA, LAMDA = 1.0, 2.0
F32 = mybir.dt.float32
BF16 = mybir.dt.bfloat16
AX = mybir.AxisListType
ALU = mybir.AluOpType
ACTF = mybir.ActivationFunctionType

NZCH = 8                       # pos stt chunks
ZCOLS = (NT * 128) // NZCH     # 1024 cols per chunk
DISP_SLOT = 26


def _build_program():
    nc = bacc.Bacc("TRN2", target_bir_lowering=False, debug=False,
                   num_devices=N_CORES)
    t = {}
    t["zT"] = nc.dram_tensor("zT", [D, B_SH], BF16, kind="ExternalInput").ap()
    t["mugT"] = nc.dram_tensor("mugT", [D, B_SH], BF16,
                               kind="ExternalInput").ap()
    t["muTs"] = nc.dram_tensor("muTs", [D, C], BF16, kind="ExternalInput").ap()
    t["muTd"] = nc.dram_tensor("muTd", [D, CD], BF16,
                               kind="ExternalInput").ap()
    t["asel"] = nc.dram_tensor("asel", [128, CD], BF16,
                               kind="ExternalInput").ap()
    t["bmask"] = nc.dram_tensor("bmask", [128, C], BF16,
                                kind="ExternalInput").ap()
    t["out"] = nc.dram_tensor("out", [128, 4], F32, kind="ExternalOutput").ap()

    with tile.TileContext(nc) as tc, ExitStack() as ctx:
        _build_tile_program(tc, ctx, t)
    nc.compile()
    return nc


def _build_tile_program(tc, ctx, t):
    nc = tc.nc
    singles = ctx.enter_context(tc.tile_pool(name="singles", bufs=1))
    scr_pool = ctx.enter_context(tc.tile_pool(name="scr", bufs=2))
    pp_pool = ctx.enter_context(tc.tile_pool(name="pp", bufs=2))
    ps_pool = ctx.enter_context(tc.tile_pool(name="ps", bufs=1, space="PSUM"))

    # Early DMAs: just what the first few tiles need.
    muTs = singles.tile([D, C], BF16)
    nc.sync.dma_start(muTs[:], t["muTs"][:, :])
    zT = singles.tile([D, B_SH], BF16)
    nc.sync.dma_start(zT[:, 0:512], t["zT"][:, 0:512])
    nc.sync.dma_start(zT[:, 512:1536], t["zT"][:, 512:1536])
    mugT = singles.tile([D, B_SH], BF16)
    muTd = singles.tile([D, CD], BF16)
    asel = singles.tile([128, CD], BF16)
    bmask = singles.tile([128, C], BF16)

    s_cols = singles.tile([128, NT + 1], F32)
    nc.vector.memset(s_cols[:], 1.0)
    nm_cols = singles.tile([128, NT + 1], F32)
    nc.vector.memset(nm_cols[:], 0.0)
    pos_cols = singles.tile([128, NZCH], F32)
    nc.vector.memset(pos_cols[:], 0.0)
    out_sb = singles.tile([128, 4], F32)
    nc.vector.memset(out_sb[:], 0.0)

    P = ps_pool.tile([128, 4096], F32)

    # Remaining DMAs, staged by the slot loop below so compute emitted in
    # between keeps its DMA-semaphore wait targets small.
    def emit_late_dma(s):
        if s == 2:
            for c in range(3):
                sl = slice(1536 + c * 2048, 1536 + (c + 1) * 2048)
                nc.sync.dma_start(zT[:, sl], t["zT"][:, sl])
        elif s == 4:
            nc.sync.dma_start(zT[:, 7680:8192], t["zT"][:, 7680:8192])
            nc.sync.dma_start(muTd[:], t["muTd"][:, :])
            nc.sync.dma_start(asel[:], t["asel"][:, :])
            nc.sync.dma_start(bmask[:], t["bmask"][:, :])
        elif s == 6:
            nc.sync.dma_start(mugT[:, 0:4096], t["mugT"][:, 0:4096])
        elif s == 8:
            nc.sync.dma_start(mugT[:, 4096:8192], t["mugT"][:, 4096:8192])

    slots = list(range(DISP_SLOT - 1)) + ["disp"] + list(range(DISP_SLOT - 1,
                                                               NT))

    def emit_mm(s):
        r = (s % 4) * 1024
        item = slots[s]
        if item == "disp":
            nc.tensor.matmul(P[0:CD, r:r + 512], muTd[:, :], muTs[:, 0:512],
                             start=True, stop=False)
            nc.tensor.matmul(P[0:CD, r:r + 512], asel[:, :], bmask[:, 0:512],
                             start=False, stop=True)
            nc.tensor.matmul(P[0:CD, r + 512:r + 1000], muTd[:, :],
                             muTs[:, 512:1000], start=True, stop=False)
            nc.tensor.matmul(P[0:CD, r + 512:r + 1000], asel[:, :],
                             bmask[:, 512:1000], start=False, stop=True)
        else:
            j = item
            lhs = zT[:, j * 128:(j + 1) * 128]
            nc.tensor.matmul(P[:, r:r + 512], lhs, muTs[:, 0:512],
                             start=True, stop=True)
            nc.tensor.matmul(P[:, r + 512:r + 1000], lhs, muTs[:, 512:1000],
                             start=True, stop=True)

    def emit_red_exp(s):
        r = (s % 4) * 1024
        item = slots[s]
        np_, col = (CD, NT) if item == "disp" else (128, item)
        nc.vector.tensor_reduce(out=nm_cols[0:np_, col:col + 1],
                                in_=P[0:np_, r:r + 1000],
                                axis=AX.X, op=ALU.max, negate=True)
        scr = scr_pool.tile([128, C], BF16, tag="scr")
        nc.scalar.activation(out=scr[0:np_, :], in_=P[0:np_, r:r + 1000],
                             func=ACTF.Exp, bias=nm_cols[0:np_, col:col + 1],
                             scale=1.0, accum_out=s_cols[0:np_, col:col + 1])

    def emit_pos_chunk(c):
        sl = slice(c * ZCOLS, (c + 1) * ZCOLS)
        pp = pp_pool.tile([128, ZCOLS], BF16, tag="pp")
        nc.vector.scalar_tensor_tensor(
            out=pp[:], in0=zT[:, sl], scalar=1.0, in1=mugT[:, sl],
            op0=ALU.mult, op1=ALU.mult, accum_out=pos_cols[:, c:c + 1])

    emit_mm(0)
    emit_mm(1)
    for s in range(len(slots)):
        emit_late_dma(s)
        if s + 2 < len(slots):
            emit_mm(s + 2)
        emit_red_exp(s)
        if s >= 32 and s % 4 == 0 and (s - 32) // 4 < NZCH:
            emit_pos_chunk((s - 32) // 4)

    # --- tail: lse rows = ln(s) - nm via the bits trick; ship partials.
    ln_cols = singles.tile([128, NT + 1], F32)
    nc.vector.tensor_scalar(
        out=ln_cols[:], in0=s_cols[:].bitcast(mybir.dt.int32),
        scalar1=8.262958405e-8, scalar2=-87.98998, op0=ALU.mult, op1=ALU.add)
    contrib = singles.tile([128, NT + 1], F32)
    nc.vector.tensor_sub(contrib[:], ln_cols[:], nm_cols[:])
    nc.vector.tensor_reduce(out=out_sb[:, 0:1], in_=contrib[:, 0:NT],
                            axis=AX.X, op=ALU.add)
    nc.vector.tensor_reduce(out=out_sb[:, 1:2], in_=pos_cols[:],
                            axis=AX.X, op=ALU.add)
    nc.vector.tensor_copy(out_sb[0:CD, 2:3], contrib[0:CD, NT:NT + 1])
    nc.sync.dma_start(t["out"][:, :], out_sb[:])


_NC_CACHE = {}


def _get_program():
    if "nc" not in _NC_CACHE:
        _NC_CACHE["nc"] = _build_program()
    return _NC_CACHE["nc"]


def make_in_maps(z, target, mu):
    import ml_dtypes
    bf16 = ml_dtypes.bfloat16
    z = np.ascontiguousarray(np.asarray(z, dtype=np.float32))
    mu = np.ascontiguousarray(np.asarray(mu, dtype=np.float32))
    target = np.asarray(target).astype(np.int64)
    muTs = np.ascontiguousarray((mu.T * np.float32(SCALE)).astype(bf16))
    muT_bf = np.ascontiguousarray(mu.T.astype(bf16))           # [128, 1000]
    asel = np.zeros((128, CD), dtype=np.float32)
    asel[np.arange(CD), np.arange(CD)] = 1.0
    asel = asel.astype(bf16)
    mu_rows10 = (mu * np.float32(SCALE)).astype(bf16)          # [1000, 128]
    in_maps = []
    for k in range(N_CORES):
        zs = z[k * B_SH:(k + 1) * B_SH]                        # [8192, 128]
        zT = np.ascontiguousarray(zs.T.astype(bf16))           # [128, 8192]
        ts = target[k * B_SH:(k + 1) * B_SH]
        mugT = np.ascontiguousarray(mu_rows10[ts].T)           # [128, 8192]
        bmask = np.zeros((128, C), dtype=np.float32)
        bmask[np.arange(CD), k * CD + np.arange(CD)] = np.float32(-1e30)
        in_maps.append({
            "zT": zT,
            "mugT": mugT,
            "muTs": muTs,
            "muTd": np.ascontiguousarray(muT_bf[:, k * CD:(k + 1) * CD]),
            "asel": asel,
            "bmask": bmask.astype(bf16),
        })
    return in_maps


def combine_outputs(results):
    outs = np.stack([np.asarray(r["out"]).astype(np.float64)
                     for r in results])                        # [8,128,4]
    comp_total = outs[:, :, 0].sum() - outs[:, :, 1].sum()
    dis_total = outs[:, :, 2].sum()
    loss_comp = comp_total / B
    loss_dis = np.log(1.0 / (C - 1)) + dis_total / C
    return np.array(ALPHA * loss_dis + LAMDA * loss_comp, dtype=np.float32)


def run_on_hw(z, target, mu, trace=False):
    nc = _get_program()
    in_maps = make_in_maps(z, target, mu)
    res = run_bass_kernel_spmd(nc, in_maps, core_ids=list(range(N_CORES)),
                               trace=trace)
    return combine_outputs(res.results), res


def kernel(z, target, mu):
    out, _ = run_on_hw(z, target, mu, trace=False)
    return out
